# revision 21
# baseline (speedup 1.0000x reference)
"""LightweightConv1dTBC forward as a Trainium2 Bass kernel.

Math: y[t, b, c] = sum_k softmax(weight)[head(c), k] * x[t + k - PAD, b, c] + bias[c]
with T=2048, B=32, C=1024, H=16 heads (R = C//H = 64 channels each), K=31, PAD=15.

Strategy:
- Hybrid shard across 8 cores: 2 time-halves x 4 batch-quarters, so each
  core owns 8 sequences over 1024 timesteps. That makes the matmul moving
  free dim 8*64 = 512 (the fp32-PSUM maximum), amortizing per-instruction
  overhead and weight loads.
- The depthwise time-conv is a banded-Toeplitz matmul on the TensorEngine:
  for each head h, a constant stationary matrix A_h[p, m] = w[h, p - m]
  (0 <= p-m < K), shape (128, 98), built on host from the softmaxed kernel.
  An input tile X of 128 consecutive timesteps (partitions) x (head, batch,
  64ch) head-major free dim yields 98 output timesteps per matmul.
- The host ships each core a zero-padded, fp16, head-major shard of
  (1108, H, 8, 64) covering its T-half plus conv halos, so the device loop
  has no boundary cases. Consecutive chunks share a 30-row halo copied
  SBUF->SBUF on the SWDGE ring instead of re-read from HBM.
- fp16 operands: 1 cycle/row matmuls, 2-byte input DMA traffic, ~3e-4
  relative error (vs 2.4e-3 for bf16). Head pairs share a 2-bank PSUM tile
  so PE wait boundaries fall every 2 matmuls and LDWEIGHTS hides under the
  previous matmul. Bank drains (98x1024) alternate vector/scalar engines.
- Input DMAs ride the sync HWDGE ring, output DMAs the scalar ring; x and y
  are (de)interleaved to/from head-major layout on the host.
"""

import numpy as np

from concourse import bacc, tile
from concourse.bass_utils import run_bass_kernel_spmd
import concourse.mybir as mybir

T, B, C, H, K, PAD = 2048, 32, 1024, 16, 31, 15
R = C // H                      # channels per head
NCORES = 8
TSH, BSH = 2, 4                 # time shards x batch shards
TL = T // TSH                   # 1024 timesteps per core
BL = B // BSH                   # 8 sequences per core
CH_IN = 128                     # input rows per chunk (partition dim)
CH_OUT = CH_IN - (K - 1)        # output rows per chunk = 98
NCH = (TL + CH_OUT - 1) // CH_OUT  # 11 chunks
NROWS = (NCH - 1) * CH_OUT + CH_IN  # 1108 shard rows incl halos/padding
HALO = 2 * PAD                  # 30 rows shared between consecutive chunks
F32 = mybir.dt.float32
F16 = mybir.dt.float16


def _build_nc(with_bias: bool):
    nc = bacc.Bacc("TRN2", target_bir_lowering=False, debug=False)
    x_d = nc.dram_tensor("x", [NROWS, H, BL, R], F16, kind="ExternalInput")
    a_d = nc.dram_tensor("a", [CH_IN, H * CH_OUT], F16, kind="ExternalInput")
    if with_bias:
        b_d = nc.dram_tensor("bias", [CH_IN, H, BL, R], F32, kind="ExternalInput")
    y_d = nc.dram_tensor("y", [TL, H, BL, R], F16, kind="ExternalOutput")

    with tile.TileContext(nc) as tc:
        with (
            tc.tile_pool(name="const", bufs=1) as cpool,
            tc.tile_pool(name="xin", bufs=6) as xpool,
            tc.tile_pool(name="yout", bufs=3) as ypool,
            tc.tile_pool(name="ps", bufs=4, space="PSUM") as pspool,
        ):
            A = cpool.tile([CH_IN, H * CH_OUT], F16)
            nc.sync.dma_start(A[:], a_d[:])
            if with_bias:
                BIAS = cpool.tile([CH_IN, H, BL, R], F32)
                nc.sync.dma_start(BIAS[:], b_d[:])

            prevX = None
            for i in range(NCH):
                t0 = i * CH_OUT
                out_m = min(CH_OUT, TL - t0)

                X = xpool.tile([CH_IN, H, BL, R], F16, tag="X")
                if i == 0:
                    nc.sync.dma_start(X[:], x_d[0:CH_IN])
                else:
                    # halo: last 30 rows of the previous tile, SBUF->SBUF on
                    # the scalar HWDGE ring (saves HBM read bandwidth)
                    nc.scalar.dma_start(X[0:HALO], prevX[CH_OUT:CH_IN])
                    nc.sync.dma_start(X[HALO:CH_IN], x_d[t0 + HALO:t0 + CH_IN])
                prevX = X

                Y = ypool.tile([CH_OUT, H, BL, R], F16, tag="Y")
                for hp in range(H // 2):   # head pairs span 2 PSUM banks
                    ps = pspool.tile([CH_OUT, 2, BL, R], F32, tag="ps")
                    for j in range(2):
                        h = 2 * hp + j
                        nc.tensor.matmul(
                            ps[:, j],
                            A[:, h * CH_OUT:(h + 1) * CH_OUT],
                            X[:, h],
                            start=True,
                            stop=True,
                        )
                    if with_bias:
                        nc.vector.tensor_tensor(
                            out=Y[0:out_m, 2 * hp:2 * hp + 2],
                            in0=ps[0:out_m],
                            in1=BIAS[0:out_m, 2 * hp:2 * hp + 2],
                            op=mybir.AluOpType.add,
                        )
                    else:
                        if hp % 2 == 0:
                            nc.vector.tensor_copy(
                                out=Y[0:out_m, 2 * hp:2 * hp + 2],
                                in_=ps[0:out_m],
                            )
                        else:
                            nc.scalar.copy(
                                out=Y[0:out_m, 2 * hp:2 * hp + 2],
                                in_=ps[0:out_m],
                            )
                nc.scalar.dma_start(y_d[t0:t0 + out_m], Y[0:out_m])

    nc.compile()
    return nc


def _toeplitz(weight: np.ndarray) -> np.ndarray:
    """Softmax the (H,1,K) kernel and build the (128, H*98) stationary matrix."""
    wl = weight[:, 0, :].astype(np.float32)
    e = np.exp(wl - wl.max(axis=-1, keepdims=True))
    w = (e / e.sum(axis=-1, keepdims=True)).astype(np.float32)  # (H, K)
    a = np.zeros((H, CH_IN, CH_OUT), dtype=np.float32)
    m = np.arange(CH_OUT)[None, :]
    p = np.arange(CH_IN)[:, None]
    k = p - m                                                   # (128, 98)
    mask = (k >= 0) & (k < K)
    for h in range(H):
        a[h][mask] = w[h][k[mask]]
    # (CH_IN, H, CH_OUT) -> head h occupies columns [h*98, (h+1)*98)
    return np.ascontiguousarray(a.transpose(1, 0, 2).reshape(CH_IN, H * CH_OUT))


def kernel(x: np.ndarray, weight: np.ndarray, bias: np.ndarray, **run_kwargs):
    x = np.ascontiguousarray(x, dtype=np.float32)
    a_all = _toeplitz(np.asarray(weight)).astype(np.float16)
    bias = np.asarray(bias, dtype=np.float32)
    with_bias = bool(np.any(bias))

    nc = _build_nc(with_bias)

    in_maps = []
    for c in range(NCORES):
        ti, bi = c // BSH, c % BSH
        # zero-padded fp16 head-major shard: row r <-> global t = ti*TL - PAD + r
        xs = np.zeros((NROWS, H, BL, R), dtype=np.float16)
        glo = ti * TL - PAD
        lo, hi = max(0, glo), min(T, glo + NROWS)
        xb = x[lo:hi, bi * BL:(bi + 1) * BL, :].reshape(hi - lo, BL, H, R)
        xs[lo - glo:hi - glo] = xb.transpose(0, 2, 1, 3).astype(np.float16)
        m = {"x": xs, "a": a_all}
        if with_bias:
            bb = np.broadcast_to(bias.reshape(H, R), (CH_IN, BL, H, R))
            m["bias"] = np.ascontiguousarray(bb.transpose(0, 2, 1, 3))
        in_maps.append(m)

    res = run_bass_kernel_spmd(nc, in_maps, core_ids=list(range(NCORES)), **run_kwargs)

    y = np.empty((T, B, C), dtype=np.float32)
    for c in range(NCORES):
        ti, bi = c // BSH, c % BSH
        # y comes back head-major (TL, H, BL, R) -> (TL, BL, C)
        yi = res.results[c]["y"].astype(np.float32).transpose(0, 2, 1, 3).reshape(TL, BL, C)
        y[ti * TL:(ti + 1) * TL, bi * BL:(bi + 1) * BL, :] = yi
    if run_kwargs:
        return y, res
    return y


# revision 22
# speedup vs baseline: 1.3457x; 1.3457x over previous
"""LightweightConv1dTBC forward as a Trainium2 Bass kernel.

Math: y[t, b, c] = sum_k softmax(weight)[head(c), k] * x[t + k - PAD, b, c] + bias[c]
with T=2048, B=32, C=1024, H=16 heads (R = C//H = 64 channels each), K=31, PAD=15.

Strategy:
- Hybrid shard across 8 cores: 2 time-halves x 4 batch-quarters, so each
  core owns 8 sequences over 1024 timesteps. That makes the matmul moving
  free dim 8*64 = 512 (the fp32-PSUM maximum), amortizing per-instruction
  overhead and weight loads.
- The depthwise time-conv is a banded-Toeplitz matmul on the TensorEngine:
  for each head h, a constant stationary matrix A_h[p, m] = w[h, p - m]
  (0 <= p-m < K), shape (128, 98), built on host from the softmaxed kernel.
  An input tile X of 128 consecutive timesteps (partitions) x (head, batch,
  64ch) head-major free dim yields 98 output timesteps per matmul.
- The host ships each core a zero-padded, fp16, head-major shard of
  (1108, H, 8, 64) covering its T-half plus conv halos, so the device loop
  has no boundary cases. Consecutive chunks share a 30-row halo copied
  SBUF->SBUF on the SWDGE ring instead of re-read from HBM.
- fp16 operands: 1 cycle/row matmuls, 2-byte input DMA traffic, ~3e-4
  relative error (vs 2.4e-3 for bf16). Head pairs share a 2-bank PSUM tile
  so PE wait boundaries fall every 2 matmuls and LDWEIGHTS hides under the
  previous matmul. Bank drains (98x1024) alternate vector/scalar engines.
- Input DMAs ride the sync HWDGE ring, output DMAs the scalar ring; x and y
  are (de)interleaved to/from head-major layout on the host.
"""

import numpy as np

from concourse import bacc, tile
from concourse.bass_utils import run_bass_kernel_spmd
import concourse.mybir as mybir

T, B, C, H, K, PAD = 2048, 32, 1024, 16, 31, 15
R = C // H                      # channels per head
NCORES = 8
TSH, BSH = 2, 4                 # time shards x batch shards
TL = T // TSH                   # 1024 timesteps per core
BL = B // BSH                   # 8 sequences per core
CH_IN = 128                     # input rows per chunk (partition dim)
CH_OUT = CH_IN - (K - 1)        # output rows per chunk = 98
NCH = (TL + CH_OUT - 1) // CH_OUT  # 11 chunks
NROWS = (NCH - 1) * CH_OUT + CH_IN  # 1108 shard rows incl halos/padding
HALO = 2 * PAD                  # 30 rows shared between consecutive chunks
F32 = mybir.dt.float32
F16 = mybir.dt.float16


def _build_nc(with_bias: bool):
    nc = bacc.Bacc("TRN2", target_bir_lowering=False, debug=False)
    x_d = nc.dram_tensor("x", [NROWS, H, BL, R], F16, kind="ExternalInput")
    a_d = nc.dram_tensor("a", [CH_IN, H * CH_OUT], F16, kind="ExternalInput")
    if with_bias:
        b_d = nc.dram_tensor("bias", [CH_IN, H, BL, R], F32, kind="ExternalInput")
    y_d = nc.dram_tensor("y", [TL, H, BL, R], F16, kind="ExternalOutput")

    with tile.TileContext(nc) as tc:
        with (
            tc.tile_pool(name="const", bufs=1) as cpool,
            tc.tile_pool(name="xin", bufs=6) as xpool,
            tc.tile_pool(name="yout", bufs=3) as ypool,
            tc.tile_pool(name="ps", bufs=4, space="PSUM") as pspool,
        ):
            A = cpool.tile([CH_IN, H * CH_OUT], F16)
            nc.sync.dma_start(A[:], a_d[:])
            if with_bias:
                BIAS = cpool.tile([CH_IN, H, BL, R], F32)
                nc.sync.dma_start(BIAS[:], b_d[:])

            for i in range(NCH):
                t0 = i * CH_OUT
                out_m = min(CH_OUT, TL - t0)

                X = xpool.tile([CH_IN, H, BL, R], F16, tag="X")
                ld = nc.sync if i % 2 == 0 else nc.scalar
                ld.dma_start(X[:], x_d[t0:t0 + CH_IN])

                Y = ypool.tile([CH_OUT, H, BL, R], F16, tag="Y")
                for hp in range(H // 2):   # head pairs span 2 PSUM banks
                    ps = pspool.tile([CH_OUT, 2, BL, R], F32, tag="ps")
                    for j in range(2):
                        h = 2 * hp + j
                        nc.tensor.matmul(
                            ps[:, j],
                            A[:, h * CH_OUT:(h + 1) * CH_OUT],
                            X[:, h],
                            start=True,
                            stop=True,
                        )
                    if with_bias:
                        nc.vector.tensor_tensor(
                            out=Y[0:out_m, 2 * hp:2 * hp + 2],
                            in0=ps[0:out_m],
                            in1=BIAS[0:out_m, 2 * hp:2 * hp + 2],
                            op=mybir.AluOpType.add,
                        )
                    else:
                        if hp % 2 == 0:
                            nc.vector.tensor_copy(
                                out=Y[0:out_m, 2 * hp:2 * hp + 2],
                                in_=ps[0:out_m],
                            )
                        else:
                            nc.scalar.copy(
                                out=Y[0:out_m, 2 * hp:2 * hp + 2],
                                in_=ps[0:out_m],
                            )
                nc.gpsimd.dma_start(y_d[t0:t0 + out_m], Y[0:out_m])

    nc.compile()
    return nc


def _toeplitz(weight: np.ndarray) -> np.ndarray:
    """Softmax the (H,1,K) kernel and build the (128, H*98) stationary matrix."""
    wl = weight[:, 0, :].astype(np.float32)
    e = np.exp(wl - wl.max(axis=-1, keepdims=True))
    w = (e / e.sum(axis=-1, keepdims=True)).astype(np.float32)  # (H, K)
    a = np.zeros((H, CH_IN, CH_OUT), dtype=np.float32)
    m = np.arange(CH_OUT)[None, :]
    p = np.arange(CH_IN)[:, None]
    k = p - m                                                   # (128, 98)
    mask = (k >= 0) & (k < K)
    for h in range(H):
        a[h][mask] = w[h][k[mask]]
    # (CH_IN, H, CH_OUT) -> head h occupies columns [h*98, (h+1)*98)
    return np.ascontiguousarray(a.transpose(1, 0, 2).reshape(CH_IN, H * CH_OUT))


def kernel(x: np.ndarray, weight: np.ndarray, bias: np.ndarray, **run_kwargs):
    x = np.ascontiguousarray(x, dtype=np.float32)
    a_all = _toeplitz(np.asarray(weight)).astype(np.float16)
    bias = np.asarray(bias, dtype=np.float32)
    with_bias = bool(np.any(bias))

    nc = _build_nc(with_bias)

    in_maps = []
    for c in range(NCORES):
        ti, bi = c // BSH, c % BSH
        # zero-padded fp16 head-major shard: row r <-> global t = ti*TL - PAD + r
        xs = np.zeros((NROWS, H, BL, R), dtype=np.float16)
        glo = ti * TL - PAD
        lo, hi = max(0, glo), min(T, glo + NROWS)
        xb = x[lo:hi, bi * BL:(bi + 1) * BL, :].reshape(hi - lo, BL, H, R)
        xs[lo - glo:hi - glo] = xb.transpose(0, 2, 1, 3).astype(np.float16)
        m = {"x": xs, "a": a_all}
        if with_bias:
            bb = np.broadcast_to(bias.reshape(H, R), (CH_IN, BL, H, R))
            m["bias"] = np.ascontiguousarray(bb.transpose(0, 2, 1, 3))
        in_maps.append(m)

    res = run_bass_kernel_spmd(nc, in_maps, core_ids=list(range(NCORES)), **run_kwargs)

    y = np.empty((T, B, C), dtype=np.float32)
    for c in range(NCORES):
        ti, bi = c // BSH, c % BSH
        # y comes back head-major (TL, H, BL, R) -> (TL, BL, C)
        yi = res.results[c]["y"].astype(np.float32).transpose(0, 2, 1, 3).reshape(TL, BL, C)
        y[ti * TL:(ti + 1) * TL, bi * BL:(bi + 1) * BL, :] = yi
    if run_kwargs:
        return y, res
    return y


# revision 23
# speedup vs baseline: 1.3951x; 1.0367x over previous
"""LightweightConv1dTBC forward as a Trainium2 Bass kernel.

Math: y[t, b, c] = sum_k softmax(weight)[head(c), k] * x[t + k - PAD, b, c] + bias[c]
with T=2048, B=32, C=1024, H=16 heads (R = C//H = 64 channels each), K=31, PAD=15.

Strategy:
- Hybrid shard across 8 cores: 2 time-halves x 4 batch-quarters, so each
  core owns 8 sequences over 1024 timesteps. That makes the matmul moving
  free dim 8*64 = 512 (the fp32-PSUM maximum), amortizing per-instruction
  overhead and weight loads.
- The depthwise time-conv is a banded-Toeplitz matmul on the TensorEngine:
  for each head h, a constant stationary matrix A_h[p, m] = w[h, p - m]
  (0 <= p-m < K), shape (128, 98), built on host from the softmaxed kernel.
  An input tile X of 128 consecutive timesteps (partitions) x (head, batch,
  64ch) head-major free dim yields 98 output timesteps per matmul.
- The host ships each core a zero-padded, fp16, head-major shard of
  (1108, H, 8, 64) covering its T-half plus conv halos, so the device loop
  has no boundary cases. Consecutive chunks share a 30-row halo copied
  SBUF->SBUF on the SWDGE ring instead of re-read from HBM.
- fp16 operands: 1 cycle/row matmuls, 2-byte input DMA traffic, ~3e-4
  relative error (vs 2.4e-3 for bf16). Head pairs share a 2-bank PSUM tile
  so PE wait boundaries fall every 2 matmuls and LDWEIGHTS hides under the
  previous matmul. Bank drains (98x1024) alternate vector/scalar engines.
- Input DMAs ride the sync HWDGE ring, output DMAs the scalar ring; x and y
  are (de)interleaved to/from head-major layout on the host.
"""

import numpy as np

from concourse import bacc, tile
from concourse.bass_utils import run_bass_kernel_spmd
import concourse.mybir as mybir

T, B, C, H, K, PAD = 2048, 32, 1024, 16, 31, 15
R = C // H                      # channels per head
NCORES = 8
TSH, BSH = 2, 4                 # time shards x batch shards
TL = T // TSH                   # 1024 timesteps per core
BL = B // BSH                   # 8 sequences per core
CH_IN = 128                     # input rows per chunk (partition dim)
CH_OUT = CH_IN - (K - 1)        # output rows per chunk = 98
NCH = (TL + CH_OUT - 1) // CH_OUT  # 11 chunks
NROWS = (NCH - 1) * CH_OUT + CH_IN  # 1108 shard rows incl halos/padding
HALO = 2 * PAD                  # 30 rows shared between consecutive chunks
F32 = mybir.dt.float32
F16 = mybir.dt.float16


def _build_nc(with_bias: bool):
    nc = bacc.Bacc("TRN2", target_bir_lowering=False, debug=False)
    x_d = nc.dram_tensor("x", [NROWS, H, BL, R], F16, kind="ExternalInput")
    a_d = nc.dram_tensor("a", [CH_IN, H * CH_OUT], F16, kind="ExternalInput")
    if with_bias:
        b_d = nc.dram_tensor("bias", [CH_IN, H, BL, R], F32, kind="ExternalInput")
    y_d = nc.dram_tensor("y", [TL, H, BL, R], F16, kind="ExternalOutput")

    with tile.TileContext(nc) as tc:
        with (
            tc.tile_pool(name="const", bufs=1) as cpool,
            tc.tile_pool(name="xin", bufs=6) as xpool,
            tc.tile_pool(name="yout", bufs=3) as ypool,
            tc.tile_pool(name="ps", bufs=4, space="PSUM") as pspool,
        ):
            A = cpool.tile([CH_IN, H * CH_OUT], F16)
            nc.sync.dma_start(A[:], a_d[:])
            if with_bias:
                BIAS = cpool.tile([CH_IN, H, BL, R], F32)
                nc.sync.dma_start(BIAS[:], b_d[:])

            for i in range(NCH):
                t0 = i * CH_OUT
                out_m = min(CH_OUT, TL - t0)

                X = xpool.tile([CH_IN, H, BL, R], F16, tag="X")
                nc.sync.dma_start(X[:], x_d[t0:t0 + CH_IN])

                Y = ypool.tile([CH_OUT, H, BL, R], F16, tag="Y")
                for hp in range(H // 2):   # head pairs span 2 PSUM banks
                    ps = pspool.tile([CH_OUT, 2, BL, R], F32, tag="ps")
                    for j in range(2):
                        h = 2 * hp + j
                        nc.tensor.matmul(
                            ps[:, j],
                            A[:, h * CH_OUT:(h + 1) * CH_OUT],
                            X[:, h],
                            start=True,
                            stop=True,
                        )
                    if with_bias:
                        nc.vector.tensor_tensor(
                            out=Y[0:out_m, 2 * hp:2 * hp + 2],
                            in0=ps[0:out_m],
                            in1=BIAS[0:out_m, 2 * hp:2 * hp + 2],
                            op=mybir.AluOpType.add,
                        )
                    else:
                        if hp % 8 < 5:
                            nc.vector.tensor_copy(
                                out=Y[0:out_m, 2 * hp:2 * hp + 2],
                                in_=ps[0:out_m],
                            )
                        else:
                            nc.scalar.copy(
                                out=Y[0:out_m, 2 * hp:2 * hp + 2],
                                in_=ps[0:out_m],
                            )
                nc.scalar.dma_start(y_d[t0:t0 + out_m], Y[0:out_m])

    nc.compile()
    return nc


def _toeplitz(weight: np.ndarray) -> np.ndarray:
    """Softmax the (H,1,K) kernel and build the (128, H*98) stationary matrix."""
    wl = weight[:, 0, :].astype(np.float32)
    e = np.exp(wl - wl.max(axis=-1, keepdims=True))
    w = (e / e.sum(axis=-1, keepdims=True)).astype(np.float32)  # (H, K)
    a = np.zeros((H, CH_IN, CH_OUT), dtype=np.float32)
    m = np.arange(CH_OUT)[None, :]
    p = np.arange(CH_IN)[:, None]
    k = p - m                                                   # (128, 98)
    mask = (k >= 0) & (k < K)
    for h in range(H):
        a[h][mask] = w[h][k[mask]]
    # (CH_IN, H, CH_OUT) -> head h occupies columns [h*98, (h+1)*98)
    return np.ascontiguousarray(a.transpose(1, 0, 2).reshape(CH_IN, H * CH_OUT))


def kernel(x: np.ndarray, weight: np.ndarray, bias: np.ndarray, **run_kwargs):
    x = np.ascontiguousarray(x, dtype=np.float32)
    a_all = _toeplitz(np.asarray(weight)).astype(np.float16)
    bias = np.asarray(bias, dtype=np.float32)
    with_bias = bool(np.any(bias))

    nc = _build_nc(with_bias)

    in_maps = []
    for c in range(NCORES):
        ti, bi = c // BSH, c % BSH
        # zero-padded fp16 head-major shard: row r <-> global t = ti*TL - PAD + r
        xs = np.zeros((NROWS, H, BL, R), dtype=np.float16)
        glo = ti * TL - PAD
        lo, hi = max(0, glo), min(T, glo + NROWS)
        xb = x[lo:hi, bi * BL:(bi + 1) * BL, :].reshape(hi - lo, BL, H, R)
        xs[lo - glo:hi - glo] = xb.transpose(0, 2, 1, 3).astype(np.float16)
        m = {"x": xs, "a": a_all}
        if with_bias:
            bb = np.broadcast_to(bias.reshape(H, R), (CH_IN, BL, H, R))
            m["bias"] = np.ascontiguousarray(bb.transpose(0, 2, 1, 3))
        in_maps.append(m)

    res = run_bass_kernel_spmd(nc, in_maps, core_ids=list(range(NCORES)), **run_kwargs)

    y = np.empty((T, B, C), dtype=np.float32)
    for c in range(NCORES):
        ti, bi = c // BSH, c % BSH
        # y comes back head-major (TL, H, BL, R) -> (TL, BL, C)
        yi = res.results[c]["y"].astype(np.float32).transpose(0, 2, 1, 3).reshape(TL, BL, C)
        y[ti * TL:(ti + 1) * TL, bi * BL:(bi + 1) * BL, :] = yi
    if run_kwargs:
        return y, res
    return y


# revision 24
# speedup vs baseline: 1.4722x; 1.0552x over previous
"""LightweightConv1dTBC forward as a Trainium2 Bass kernel.

Math: y[t, b, c] = sum_k softmax(weight)[head(c), k] * x[t + k - PAD, b, c] + bias[c]
with T=2048, B=32, C=1024, H=16 heads (R = C//H = 64 channels each), K=31, PAD=15.

Strategy:
- Hybrid shard across 8 cores: 2 time-halves x 4 batch-quarters, so each
  core owns 8 sequences over 1024 timesteps. That makes the matmul moving
  free dim 8*64 = 512 (the fp32-PSUM maximum), amortizing per-instruction
  overhead and weight loads.
- The depthwise time-conv is a banded-Toeplitz matmul on the TensorEngine:
  for each head h, a constant stationary matrix A_h[p, m] = w[h, p - m]
  (0 <= p-m < K), shape (128, 98), built on host from the softmaxed kernel.
  An input tile X of 128 consecutive timesteps (partitions) x (head, batch,
  64ch) head-major free dim yields 98 output timesteps per matmul.
- The host ships each core a zero-padded, fp16, head-major shard of
  (1108, H, 8, 64) covering its T-half plus conv halos, so the device loop
  has no boundary cases. Consecutive chunks share a 30-row halo copied
  SBUF->SBUF on the SWDGE ring instead of re-read from HBM.
- fp16 operands: 1 cycle/row matmuls, 2-byte input DMA traffic, ~3e-4
  relative error (vs 2.4e-3 for bf16). Head pairs share a 2-bank PSUM tile
  so PE wait boundaries fall every 2 matmuls and LDWEIGHTS hides under the
  previous matmul. Bank drains (98x1024) alternate vector/scalar engines.
- Input DMAs ride the sync HWDGE ring, output DMAs the scalar ring; x and y
  are (de)interleaved to/from head-major layout on the host.
"""

import numpy as np

from concourse import bacc, tile
from concourse.bass_utils import run_bass_kernel_spmd
import concourse.mybir as mybir

T, B, C, H, K, PAD = 2048, 32, 1024, 16, 31, 15
R = C // H                      # channels per head
NCORES = 8
TSH, BSH = 2, 4                 # time shards x batch shards
TL = T // TSH                   # 1024 timesteps per core
BL = B // BSH                   # 8 sequences per core
CH_IN = 128                     # input rows per chunk (partition dim)
CH_OUT = CH_IN - (K - 1)        # output rows per chunk = 98
NCH = (TL + CH_OUT - 1) // CH_OUT  # 11 chunks
NROWS = (NCH - 1) * CH_OUT + CH_IN  # 1108 shard rows incl halos/padding
HALO = 2 * PAD                  # 30 rows shared between consecutive chunks
F32 = mybir.dt.float32
F16 = mybir.dt.float16


def _build_nc(with_bias: bool):
    nc = bacc.Bacc("TRN2", target_bir_lowering=False, debug=False)
    x_d = nc.dram_tensor("x", [NROWS, H, BL, R], F16, kind="ExternalInput")
    a_d = nc.dram_tensor("a", [CH_IN, H * CH_OUT], F16, kind="ExternalInput")
    if with_bias:
        b_d = nc.dram_tensor("bias", [CH_IN, H, BL, R], F32, kind="ExternalInput")
    y_d = nc.dram_tensor("y", [TL, H, BL, R], F16, kind="ExternalOutput")

    with tile.TileContext(nc) as tc:
        with (
            tc.tile_pool(name="const", bufs=1) as cpool,
            tc.tile_pool(name="xin", bufs=6) as xpool,
            tc.tile_pool(name="yout", bufs=3) as ypool,
            tc.tile_pool(name="ps", bufs=4, space="PSUM") as pspool,
        ):
            A = cpool.tile([CH_IN, H * CH_OUT], F16)
            nc.sync.dma_start(A[:], a_d[:])
            if with_bias:
                BIAS = cpool.tile([CH_IN, H, BL, R], F32)
                nc.sync.dma_start(BIAS[:], b_d[:])

            for i in range(NCH):
                t0 = i * CH_OUT
                out_m = min(CH_OUT, TL - t0)

                X = xpool.tile([CH_IN, H, BL, R], F16, tag="X")
                nc.sync.dma_start(X[:], x_d[t0:t0 + CH_IN])

                Y = ypool.tile([CH_OUT, H, BL, R], F16, tag="Y")
                for hp in range(H // 2):   # head pairs span 2 PSUM banks
                    ps = pspool.tile([CH_OUT, 2, BL, R], F32, tag="ps")
                    for j in range(2):
                        h = 2 * hp + j
                        nc.tensor.matmul(
                            ps[:, j],
                            A[:, h * CH_OUT:(h + 1) * CH_OUT],
                            X[:, h],
                            start=True,
                            stop=True,
                        )
                    if with_bias:
                        nc.vector.tensor_tensor(
                            out=Y[0:out_m, 2 * hp:2 * hp + 2],
                            in0=ps[0:out_m],
                            in1=BIAS[0:out_m, 2 * hp:2 * hp + 2],
                            op=mybir.AluOpType.add,
                        )
                    else:
                        if hp % 8 >= 5:
                            nc.vector.tensor_copy(
                                out=Y[0:out_m, 2 * hp:2 * hp + 2],
                                in_=ps[0:out_m],
                            )
                        else:
                            nc.scalar.copy(
                                out=Y[0:out_m, 2 * hp:2 * hp + 2],
                                in_=ps[0:out_m],
                            )
                nc.scalar.dma_start(y_d[t0:t0 + out_m], Y[0:out_m])

    nc.compile()
    return nc


def _toeplitz(weight: np.ndarray) -> np.ndarray:
    """Softmax the (H,1,K) kernel and build the (128, H*98) stationary matrix."""
    wl = weight[:, 0, :].astype(np.float32)
    e = np.exp(wl - wl.max(axis=-1, keepdims=True))
    w = (e / e.sum(axis=-1, keepdims=True)).astype(np.float32)  # (H, K)
    a = np.zeros((H, CH_IN, CH_OUT), dtype=np.float32)
    m = np.arange(CH_OUT)[None, :]
    p = np.arange(CH_IN)[:, None]
    k = p - m                                                   # (128, 98)
    mask = (k >= 0) & (k < K)
    for h in range(H):
        a[h][mask] = w[h][k[mask]]
    # (CH_IN, H, CH_OUT) -> head h occupies columns [h*98, (h+1)*98)
    return np.ascontiguousarray(a.transpose(1, 0, 2).reshape(CH_IN, H * CH_OUT))


def kernel(x: np.ndarray, weight: np.ndarray, bias: np.ndarray, **run_kwargs):
    x = np.ascontiguousarray(x, dtype=np.float32)
    a_all = _toeplitz(np.asarray(weight)).astype(np.float16)
    bias = np.asarray(bias, dtype=np.float32)
    with_bias = bool(np.any(bias))

    nc = _build_nc(with_bias)

    in_maps = []
    for c in range(NCORES):
        ti, bi = c // BSH, c % BSH
        # zero-padded fp16 head-major shard: row r <-> global t = ti*TL - PAD + r
        xs = np.zeros((NROWS, H, BL, R), dtype=np.float16)
        glo = ti * TL - PAD
        lo, hi = max(0, glo), min(T, glo + NROWS)
        xb = x[lo:hi, bi * BL:(bi + 1) * BL, :].reshape(hi - lo, BL, H, R)
        xs[lo - glo:hi - glo] = xb.transpose(0, 2, 1, 3).astype(np.float16)
        m = {"x": xs, "a": a_all}
        if with_bias:
            bb = np.broadcast_to(bias.reshape(H, R), (CH_IN, BL, H, R))
            m["bias"] = np.ascontiguousarray(bb.transpose(0, 2, 1, 3))
        in_maps.append(m)

    res = run_bass_kernel_spmd(nc, in_maps, core_ids=list(range(NCORES)), **run_kwargs)

    y = np.empty((T, B, C), dtype=np.float32)
    for c in range(NCORES):
        ti, bi = c // BSH, c % BSH
        # y comes back head-major (TL, H, BL, R) -> (TL, BL, C)
        yi = res.results[c]["y"].astype(np.float32).transpose(0, 2, 1, 3).reshape(TL, BL, C)
        y[ti * TL:(ti + 1) * TL, bi * BL:(bi + 1) * BL, :] = yi
    if run_kwargs:
        return y, res
    return y
